# revision 10
# baseline (speedup 1.0000x reference)
"""Sparse cross-attention kernel for Trainium2 (8 NeuronCores).

Sharding: valid tokens (rows of the NxN attention) are sharded across the 8
cores -- each core holds 1024 queries and the full gathered key/value set
(8192 tokens), computes its energy rows + softmax + output rows.

Device layout trick: energy is computed TRANSPOSED (eT[key, query], keys on
partitions) so that
  * the exp for softmax is a single ScalarE pass PSUM->SBUF (it doubles as
    the PSUM-evacuation copy),
  * the attention matmul consumes exp(eT) directly as the moving operand with
    token-major yt tiles as stationary weights (no transposes anywhere),
  * a ones-column appended to yt makes the softmax denominator fall out of
    the same accumulation for free.
The value/output projections fold into a single tiny (Wz @ Wt) matrix that is
applied on the host, together with the global group-norm statistics (which
need all 8192 tokens), the scatter back into the dense map, and the residual.
"""

import sys

import numpy as np

sys.path.insert(0, "/opt/trn_rl_repo")

import concourse.bacc as bacc  # noqa: E402
import concourse.tile as tile  # noqa: E402
from concourse import bass_utils, mybir  # noqa: E402

# problem constants (hardcoded per contract)
B, CQ, CK, F, H, W = 2, 32, 16, 64, 128, 128
NV = 8192               # number of valid (mask > 0) tokens
NCORES = 8
QL = NV // NCORES       # queries per core
KB = 128                # key block (partition dim of eT tiles)
NKB = NV // KB          # 64 key blocks
CA = CK + 1             # yt channels + ones column
NQMM = QL // 512        # moving-dim chunks per matmul (fp32 max free 512)
EPS = 1e-5

FP32 = mybir.dt.float32
FP32R = mybir.dt.float32r
BF16 = mybir.dt.bfloat16

USE_FP32R = True        # big matmuls in fp32r (1 cyc/row vs 4 for fp32)
MMDT = FP32R if USE_FP32R else FP32  # dtype of tiles feeding the energy matmul
# the attention matmul runs in bf16: fp32r forbids dst partitions != 0, which
# would kill column packing; bf16 error only enters via attention weights
ATTDT = BF16
COLPACK = True          # 2-way column packing of the attention matmul

_PROG = None
LAST_RESULTS = None     # BassKernelResults of the last run (for test harness)


def _body(tc, d_xtT, d_ytT, d_ytaug, d_wpT, d_wgT, d_out, niters=1):
    nc = tc.nc
    from contextlib import ExitStack

    with ExitStack() as ctx:
        const = ctx.enter_context(tc.tile_pool(name="const", bufs=1))
        xpool = ctx.enter_context(tc.tile_pool(name="xp", bufs=8))
        epool = ctx.enter_context(tc.tile_pool(name="ep", bufs=3, space="PSUM"))
        apool = ctx.enter_context(tc.tile_pool(name="acc", bufs=1, space="PSUM"))
        for _it in range(niters):
            _iter(nc, const, xpool, epool, apool,
                  d_xtT, d_ytT, d_ytaug, d_wpT, d_wgT, d_out)


def _iter(nc, const, xpool, epool, apool,
          d_xtT, d_ytT, d_ytaug, d_wpT, d_wgT, d_out):
    if True:

        # ---- input DMAs ----
        wp_s = const.tile([CQ, F], FP32)
        nc.sync.dma_start(out=wp_s[:], in_=d_wpT[:])
        wg_s = const.tile([CK, F], FP32)
        nc.sync.dma_start(out=wg_s[:], in_=d_wgT[:])
        xtT_s = const.tile([CQ, QL], FP32)
        nc.sync.dma_start(out=xtT_s[:], in_=d_xtT[:])
        # chunk the key DMA so the g-projection (and thus the main loop) can
        # start as soon as the first slice lands
        ytT_s = const.tile([CK, NV], FP32)
        for c in range(16):
            nc.sync.dma_start(
                out=ytT_s[:, c * 512:(c + 1) * 512],
                in_=d_ytT[:, c * 512:(c + 1) * 512],
            )
        ytaug_s = const.tile([128, NKB * CA], ATTDT)
        nc.sync.dma_start(out=ytaug_s[:], in_=d_ytaug[:])

        # ---- query projection: pT[f, q] = Wp @ xtT ----
        p_ps = epool.tile([F, QL], FP32, tag="et")
        for i in range(NQMM):
            nc.tensor.matmul(
                out=p_ps[:, i * 512:(i + 1) * 512],
                lhsT=wp_s[:],
                rhs=xtT_s[:, i * 512:(i + 1) * 512],
                start=True, stop=True,
            )
        pT_s = const.tile([F, QL], MMDT)
        nc.vector.tensor_copy(out=pT_s[:], in_=p_ps[:])

        # ---- key projection: gT[f, k] = Wg @ ytT ----
        gT_s = const.tile([F, NV], MMDT)
        for c in range(16):
            g_ps = epool.tile([F, 512], FP32, tag="et")
            nc.tensor.matmul(
                out=g_ps[:],
                lhsT=wg_s[:],
                rhs=ytT_s[:, c * 512:(c + 1) * 512],
                start=True, stop=True,
            )
            nc.vector.tensor_copy(out=gT_s[:, c * 512:(c + 1) * 512], in_=g_ps[:])

        # ---- attention accumulator: [yt | 1].T @ exp(eT), [17, QL] per
        # column group. The two groups share PSUM banks at disjoint partition
        # ranges (0:17 and 64:81) -- the standard col-tiling layout; PSUM
        # has_written bits are per-element so the interleaved accumulation
        # groups don't interact (the sim's coarse zero-region tracker can't
        # see that, hence skip_group_check on the matmuls). ----
        outS_ps = apool.tile([128, QL], FP32)

        for j in range(NKB):
            # energy block (transposed): eT[k, q] = g_k . p_q
            e_ps = epool.tile([128, QL], FP32, tag="et")
            lhs_g = gT_s[:, j * KB:(j + 1) * KB]
            rhs_p = pT_s[:]
            for i in range(NQMM):
                nc.tensor.matmul(
                    out=e_ps[:, i * 512:(i + 1) * 512],
                    lhsT=lhs_g,
                    rhs=rhs_p[:, i * 512:(i + 1) * 512],
                    start=True, stop=True,
                )
            # softmax numerator: exp straight out of PSUM into SBUF.
            # No max subtraction: energies are O(+-50), well inside fp32 exp
            # range, and the reference's max-shift cancels mathematically.
            x_s = xpool.tile([128, QL], ATTDT, tag="xp")
            nc.scalar.activation(
                out=x_s[:], in_=e_ps[:], func=mybir.ActivationFunctionType.Exp
            )
            # attention matmul, accumulated over key blocks; even/odd blocks
            # go to different PE column groups (disjoint PSUM partitions) so
            # consecutive blocks run concurrently on the array
            grp = (j % 2) if COLPACK else 0
            bp = 64 * grp
            lhs_t = ytaug_s[:, j * CA:(j + 1) * CA]
            rhs_x = x_s[:]
            if COLPACK:
                start = j == grp
                stop = j == (NKB - 2 + grp)
            else:
                start = j == 0
                stop = j == NKB - 1
            for i in range(NQMM):
                nc.tensor.matmul(
                    out=outS_ps[bp:bp + CA, i * 512:(i + 1) * 512],
                    lhsT=lhs_t,
                    rhs=rhs_x[:, i * 512:(i + 1) * 512],
                    start=start, stop=stop,
                    tile_position=(0, bp) if COLPACK else None,
                    skip_group_check=COLPACK,
                )

        # ---- evacuate the two accumulator slices and DMA out ----
        sbf = const.tile([128, QL], FP32)
        nc.vector.tensor_copy(out=sbf[0:CA, :], in_=outS_ps[0:CA, :])
        nc.sync.dma_start(out=d_out[0:CA, :], in_=sbf[0:CA, :])
        if COLPACK:
            nc.vector.tensor_copy(
                out=sbf[64:64 + CA, :], in_=outS_ps[64:64 + CA, :]
            )
            nc.sync.dma_start(out=d_out[CA:2 * CA, :], in_=sbf[64:64 + CA, :])


def build_program(niters=1):
    nc = bacc.Bacc(
        "TRN2", target_bir_lowering=False, debug=False, num_devices=NCORES
    )
    d_xtT = nc.dram_tensor("xtT", [CQ, QL], FP32, kind="ExternalInput").ap()
    d_ytT = nc.dram_tensor("ytT", [CK, NV], FP32, kind="ExternalInput").ap()
    d_ytaug = nc.dram_tensor(
        "ytaug", [128, NKB * CA], ATTDT, kind="ExternalInput"
    ).ap()
    d_wpT = nc.dram_tensor("wpT", [CQ, F], FP32, kind="ExternalInput").ap()
    d_wgT = nc.dram_tensor("wgT", [CK, F], FP32, kind="ExternalInput").ap()
    d_out = nc.dram_tensor(
        "outS2", [2 * CA, QL], FP32, kind="ExternalOutput"
    ).ap()

    with tile.TileContext(nc) as tc:
        _body(tc, d_xtT, d_ytT, d_ytaug, d_wpT, d_wgT, d_out, niters=niters)
    nc.compile()
    return nc


def get_program():
    global _PROG
    if _PROG is None:
        _PROG = build_program()
    return _PROG


def make_in_maps(x, y, masks):
    """Host-side sharding: gather the valid tokens, lay them out for the
    device, and split the queries across the 8 cores."""
    x = np.ascontiguousarray(np.asarray(x, dtype=np.float32))
    y = np.ascontiguousarray(np.asarray(y, dtype=np.float32))
    mflat = np.asarray(masks).reshape(-1)
    idx = np.flatnonzero(mflat > 0)
    assert idx.size == NV, f"expected {NV} valid tokens, got {idx.size}"

    xt = x.transpose(0, 2, 3, 1).reshape(-1, CQ)[idx]            # [NV, CQ]
    yt = y.transpose(0, 2, 3, 1).reshape(-1, CK)[idx]            # [NV, CK]
    ytT = np.ascontiguousarray(yt.T)                             # [CK, NV]
    ytaug = np.concatenate(
        [yt, np.ones((NV, 1), np.float32)], axis=1
    )                                                            # [NV, CA]
    import ml_dtypes
    ytaug_dev = np.ascontiguousarray(
        ytaug.reshape(NKB, 128, CA).transpose(1, 0, 2).reshape(128, NKB * CA)
    ).astype(ml_dtypes.bfloat16)
    in_maps = []
    for c in range(NCORES):
        xtT_c = np.ascontiguousarray(xt[c * QL:(c + 1) * QL].T)  # [CQ, QL]
        in_maps.append({
            "xtT": xtT_c,
            "ytT": ytT,
            "ytaug": ytaug_dev,
            "wpT": None,  # filled by caller (weights)
            "wgT": None,
        })
    return idx, in_maps


def kernel(x, y, masks, Wp, Wt, Wg, Wz, gn_w, gn_b, trace=False):
    global LAST_RESULTS
    x = np.ascontiguousarray(np.asarray(x, dtype=np.float32))
    Wp = np.asarray(Wp, dtype=np.float32)
    Wt = np.asarray(Wt, dtype=np.float32)
    Wg = np.asarray(Wg, dtype=np.float32)
    Wz = np.asarray(Wz, dtype=np.float32)
    gn_w = np.asarray(gn_w, dtype=np.float32)
    gn_b = np.asarray(gn_b, dtype=np.float32)

    idx, in_maps = make_in_maps(x, y, masks)
    wpT = np.ascontiguousarray(Wp.T)                             # [CQ, F]
    wgT = np.ascontiguousarray(Wg.T)                             # [CK, F]
    for m in in_maps:
        m["wpT"] = wpT
        m["wgT"] = wgT

    nc = get_program()
    try:
        res = bass_utils.run_bass_kernel_spmd(
            nc, in_maps, core_ids=list(range(NCORES)), trace=trace
        )
    except ModuleNotFoundError:
        # NTFF profiling hook unavailable in this environment
        res = bass_utils.run_bass_kernel_spmd(
            nc, in_maps, core_ids=list(range(NCORES)), trace=False
        )
    LAST_RESULTS = res

    # ---- host-side unshard: normalize, fold value/output projection,
    # global group norm, scatter, residual ----
    Wzt = Wz.astype(np.float64) @ Wt.astype(np.float64)          # [CQ, CK]
    z = np.empty((NV, CQ), np.float32)
    for c in range(NCORES):
        o = res.results[c]["outS2"].astype(np.float64)           # [2*CA, QL]
        outS = o[0:CA] + o[CA:2 * CA] if COLPACK else o[0:CA]    # [CA, QL]
        att = outS[0:CK] / outS[CK:CK + 1]                       # [CK, QL]
        z[c * QL:(c + 1) * QL] = (Wzt @ att).T.astype(np.float32)
    zd = z.astype(np.float64)
    mu = zd.mean()
    var = zd.var()
    zn = ((zd - mu) / np.sqrt(var + EPS)).astype(np.float32)
    zn = zn * gn_w[None, :] + gn_b[None, :]
    out_tok = np.zeros((B * H * W, CQ), np.float32)
    out_tok[idx] = zn
    out = out_tok.reshape(B, H * W, CQ).transpose(0, 2, 1).reshape(B, CQ, H, W)
    return out + x
